# revision 16
# baseline (speedup 1.0000x reference)
"""Trainium2 Bass kernel for AdaptiveNoisingModule (retrieval kNN).

Math (matches the jax reference):
  f = features (B,C,H,W) -> (B*H*W, C) query rows
  d2[n,m] = |f_n|^2 + |mb_m|^2 - 2 f_n.mb_m ; nearest = argmin_m d2
  influence[n] = mean_c |f_n - mb_nearest| / (sqrt(clip(d2min,0)+1e-8) + 1e-8)
  influence_norm = (influence - min) / (max - min)   [global min/max]
  noise_std = 0.01 + 0.49 * influence_norm
  noised = f + noise * noise_std      (noise = jax.random.normal(key(1)))

Sharding: data-parallel over queries; batch image i -> core i (784 rows each).
Memory bank replicated. Only the global min/max of influence is all-reduced.

Device algorithm per core:
  score[n,m] = 2 f.mb - |mb|^2  computed as ONE f32r matmul by extending the
  contraction dim: lhsT rows 0..1023 = 2 f^T, rows 1024/1025 = 1.0;
  rhs rows 0..1023 = mb^T, row 1024 = -mnorm_hi, row 1025 = -mnorm_lo
  (hi/lo split because f32r rounds operands to 11 mantissa bits).
  argmin_m d2 = argmax_m score. Row-local argmax via DVE MAX8/MAX_INDEX over
  1024-wide m-groups, then a tiny cross-group combine. Nearest rows fetched
  with an indirect (gather) DMA; influence on DVE/ACT; min/max AllReduce
  across the 8 cores; noising fused with scalar_tensor_tensor.
"""

import numpy as np

B, C, H, W = 8, 1024, 28, 28
M = 16384
N_CORES = 8
NQ = H * W          # 784 queries per core (batch-sharded)
CE = 1152           # padded contraction: 1024 + 2 norm rows + zeros
KB = CE // 128      # 9 k-blocks
G = 16              # m-groups
GW = M // G         # 1024 group width
NT = GW // 512      # psum tiles per group
P = 128
NB = (NQ + P - 1) // P          # 7 row blocks (6x128 + 16)
BLK = [(i * P, min(P, NQ - i * P)) for i in range(NB)]
EPS = 1e-8
NOISE_MIN, NOISE_MAX = 0.01, 0.5
BIG = 3.0e7

_CACHE = {}


def build_bass(stage=4):
    """Build the SPMD Bass graph (same program for all 8 cores).

    stage: debug bisection level — 1: matmul+argmax only, 2: +gather+influence,
    3: +collective, 4: full (normalize+noise outputs).
    """
    if ("nc", stage) in _CACHE:
        return _CACHE[("nc", stage)]
    from contextlib import ExitStack
    import concourse.bass as bass
    import concourse.tile as tile
    from concourse import bacc, mybir

    f32 = mybir.dt.float32
    f32r = mybir.dt.float32r
    u32 = mybir.dt.uint32
    AX = mybir.AxisListType
    OP = mybir.AluOpType
    ACTF = mybir.ActivationFunctionType

    nc = bacc.Bacc("TRN2", target_bir_lowering=False, debug=False,
                   num_devices=N_CORES)

    ft = nc.declare_dram_parameter("ft", [CE, NQ], f32r, isOutput=False)
    mbt = nc.declare_dram_parameter("mbt", [CE, M], f32r, isOutput=False)
    fq = nc.declare_dram_parameter("fq", [NQ, C], f32, isOutput=False)
    mbrows = nc.declare_dram_parameter("mbrows", [M, C], f32, isOutput=False)
    noise = nc.declare_dram_parameter("noise", [NQ, C], f32, isOutput=False)
    out_x = nc.declare_dram_parameter("out_noised", [NQ, C], f32, isOutput=True)
    out_i = nc.declare_dram_parameter("out_inf", [NQ, 1], f32, isOutput=True)
    out_s = nc.declare_dram_parameter("out_std", [NQ, 1], f32, isOutput=True)

    with tile.TileContext(nc) as tc, ExitStack() as ctx:
        const = ctx.enter_context(tc.tile_pool(name="const", bufs=1))
        mbtp = ctx.enter_context(tc.tile_pool(name="mbtp", bufs=2))
        scp = ctx.enter_context(tc.tile_pool(name="scores", bufs=3))
        psp = ctx.enter_context(tc.tile_pool(name="psum", bufs=4, space="PSUM"))
        wk = ctx.enter_context(tc.tile_pool(name="work", bufs=2))
        sm = ctx.enter_context(tc.tile_pool(name="small", bufs=2))
        drp = ctx.enter_context(tc.tile_pool(name="dram", bufs=1, space="DRAM"))

        # ---- resident tiles ----
        ft_sb = const.tile([P, KB, NQ], f32r)
        nc.sync.dma_start(ft_sb[:], ft.rearrange("(kb p) n -> p kb n", p=P))

        gval = const.tile([P, NB, G, 8], f32)   # top-8 scores per (block, group)
        gidx = const.tile([P, NB, G, 8], u32)   # their in-group indices

        goff = const.tile([P, G], f32)          # column g -> g*GW
        for g in range(G):
            nc.vector.memset(goff[:, g:g + 1], float(g * GW))
        bigc = const.tile([P, G], f32)
        nc.vector.memset(bigc[:], BIG)

        epsc = const.tile([P, 1], f32)
        nc.vector.memset(epsc[:], EPS)

        inf_cols = const.tile([P, NB], f32)     # influence per block column
        infmin = const.tile([P, NB], f32)       # +BIG filled (for min)
        infmax = const.tile([P, NB], f32)       # -BIG filled (for max)
        nc.vector.memset(infmin[:], BIG)
        nc.vector.memset(infmax[:], -BIG)

        # ---- main loop: scores + per-group argmax ----
        for g in range(G):
            mbt_g = mbtp.tile([P, KB, GW], f32r)
            nc.sync.dma_start(
                mbt_g[:],
                mbt[:, g * GW:(g + 1) * GW].rearrange("(kb p) m -> p kb m", p=P),
            )
            for b, (n0, pb) in enumerate(BLK):
                sc = scp.tile([P, GW], f32)
                for t in range(NT):
                    ps = psp.tile([P, 512], f32)
                    for kb in range(KB):
                        nc.tensor.matmul(
                            ps[:pb],
                            ft_sb[:, kb, n0:n0 + pb],
                            mbt_g[:, kb, t * 512:(t + 1) * 512],
                            start=(kb == 0),
                            stop=(kb == KB - 1),
                        )
                    nc.scalar.copy(sc[:pb, t * 512:(t + 1) * 512], ps[:pb])
                nc.vector.max(gval[:pb, b, g, :], sc[:pb, :])
                nc.vector.max_index(gidx[:pb, b, g, :], gval[:pb, b, g, :],
                                    sc[:pb, :])

        # ---- per-block: combine groups, gather nearest, influence ----
        for b, (n0, pb) in enumerate(BLK if stage >= 2 else []):
            vals = gval[:pb, b, :, 0]            # [pb, G] stride-8
            maxv = sm.tile([P, 1], f32)
            nc.vector.tensor_reduce(maxv[:pb], vals, axis=AX.X, op=OP.max)
            mask = sm.tile([P, G], u32)
            nc.vector.tensor_scalar(mask[:pb], vals, maxv[:pb], None,
                                    op0=OP.is_ge)
            idxf = sm.tile([P, G], f32)
            nc.vector.tensor_copy(idxf[:pb], gidx[:pb, b, :, 0])  # u32 -> f32
            nc.vector.tensor_tensor(idxf[:pb], idxf[:pb], goff[:pb], op=OP.add)
            cand = sm.tile([P, G], f32)
            nc.vector.select(cand[:pb], mask[:pb], idxf[:pb], bigc[:pb])
            idxm = sm.tile([P, 1], f32)
            nc.vector.tensor_reduce(idxm[:pb], cand[:pb], axis=AX.X, op=OP.min)
            idxu = sm.tile([P, 1], u32)
            nc.vector.tensor_copy(idxu[:pb], idxm[:pb])           # f32 -> u32

            nn = wk.tile([P, C], f32, tag="nn")
            nc.gpsimd.indirect_dma_start(
                nn[:pb], None, mbrows[:],
                bass.IndirectOffsetOnAxis(ap=idxu[:pb], axis=0),
            )
            fqb = wk.tile([P, C], f32, tag="fqb")
            nc.sync.dma_start(fqb[:pb], fq[n0:n0 + pb, :])

            if stage == 2:
                dbg = wk.tile([P, C], f32, tag="ox")
                nc.vector.memset(dbg[:], 0.0)
                nc.vector.tensor_copy(dbg[:pb, 0:1], idxm[:pb])
                nc.vector.tensor_copy(dbg[:pb, 1:2], maxv[:pb])
                nc.vector.tensor_copy(dbg[:pb, 8:8 + 64], nn[:pb, 0:64])
                nc.sync.dma_start(out_x[n0:n0 + pb, :], dbg[:pb])
                continue

            scr = wk.tile([P, C], f32, tag="scr")
            qn = sm.tile([P, 1], f32)
            nc.scalar.activation(scr[:pb], fqb[:pb], ACTF.Square,
                                 accum_out=qn[:pb])
            if stage == 21:
                dbg = wk.tile([P, C], f32, tag="ox")
                nc.vector.memset(dbg[:], 0.0)
                nc.vector.tensor_copy(dbg[:pb, 0:1], qn[:pb])
                nc.sync.dma_start(out_x[n0:n0 + pb, :], dbg[:pb])
                continue
            diff = wk.tile([P, C], f32, tag="diff")
            nc.vector.tensor_tensor(diff[:pb], fqb[:pb], nn[:pb], op=OP.subtract)
            asum = sm.tile([P, 1], f32)
            nc.vector.tensor_reduce(asum[:pb], diff[:pb], axis=AX.X, op=OP.add,
                                    apply_absolute_value=True)
            if stage == 22:
                dbg = wk.tile([P, C], f32, tag="ox")
                nc.vector.memset(dbg[:], 0.0)
                nc.vector.tensor_copy(dbg[:pb, 0:1], qn[:pb])
                nc.vector.tensor_copy(dbg[:pb, 1:2], asum[:pb])
                nc.sync.dma_start(out_x[n0:n0 + pb, :], dbg[:pb])
                continue
            d2 = sm.tile([P, 1], f32)
            nc.vector.tensor_tensor(d2[:pb], qn[:pb], maxv[:pb], op=OP.subtract)
            nc.vector.tensor_scalar_max(d2[:pb], d2[:pb], 0.0)
            dist = sm.tile([P, 1], f32)
            nc.scalar.activation(dist[:pb], d2[:pb], ACTF.Sqrt, bias=epsc[:pb])
            den = sm.tile([P, 1], f32)
            nc.vector.tensor_scalar_add(den[:pb], dist[:pb], EPS)
            rden = sm.tile([P, 1], f32)
            nc.vector.reciprocal(rden[:pb], den[:pb])
            infb = sm.tile([P, 1], f32)
            nc.vector.tensor_tensor(infb[:pb], asum[:pb], rden[:pb], op=OP.mult)
            nc.vector.tensor_scalar_mul(infb[:pb], infb[:pb], 1.0 / C)
            if stage == 23:
                dbg = wk.tile([P, C], f32, tag="ox")
                nc.vector.memset(dbg[:], 0.0)
                nc.vector.tensor_copy(dbg[:pb, 0:1], infb[:pb])
                nc.sync.dma_start(out_x[n0:n0 + pb, :], dbg[:pb])
                continue
            nc.vector.tensor_copy(inf_cols[:pb, b:b + 1], infb[:pb])
            nc.vector.tensor_copy(infmin[:pb, b:b + 1], infb[:pb])
            nc.vector.tensor_copy(infmax[:pb, b:b + 1], infb[:pb])

        if stage < 2:
            # debug: just dump raw top scores/indices
            for b, (n0, pb) in enumerate(BLK):
                dbg = wk.tile([P, C], f32, tag="ox")
                nc.vector.memset(dbg[:], 0.0)
                nc.vector.tensor_copy(dbg[:pb, 0:G * 8],
                                      gval[:pb, b, :, :].rearrange("p g e -> p (g e)"))
                nc.vector.tensor_copy(dbg[:pb, 256:256 + G * 8],
                                      gidx[:pb, b, :, :].rearrange("p g e -> p (g e)"))
                nc.sync.dma_start(out_x[n0:n0 + pb, :], dbg[:pb])

        if stage in (3, 4):
            # ---- global min/max: partition-reduce via DRAM, AllReduce ----
            vmax = sm.tile([P, 1], f32)
            nc.vector.tensor_reduce(vmax[:], infmax[:], axis=AX.X, op=OP.max)
            vmin = sm.tile([P, 1], f32)
            nc.vector.tensor_reduce(vmin[:], infmin[:], axis=AX.X, op=OP.min)
            pk = sm.tile([P, 2], f32)
            nc.vector.tensor_copy(pk[:, 0:1], vmax[:])
            nc.vector.tensor_scalar_mul(pk[:, 1:2], vmin[:], -1.0)  # -min

            dr_pack = drp.tile([1, 2 * P], f32)
            nc.sync.dma_start(dr_pack[:].rearrange("o (p c) -> (o p) c", c=2),
                              pk[:])
            flat = sm.tile([1, 2 * P], f32)
            nc.sync.dma_start(flat[:], dr_pack[:])
            red2 = sm.tile([1, 2], f32)
            nc.vector.tensor_reduce(
                red2[:], flat[0:1].rearrange("o (p c) -> o c p", c=2),
                axis=AX.X, op=OP.max,
            )
            g2 = sm.tile([P, 2], f32)
            if stage >= 4:
                cc_in = drp.tile([1, 2], f32)
                nc.sync.dma_start(cc_in[:], red2[:])
                cc_out = drp.tile([1, 2], f32, addr_space="Shared")
                nc.gpsimd.collective_compute(
                    "AllReduce", OP.max,
                    replica_groups=[list(range(N_CORES))],
                    ins=[cc_in[:].opt()],
                    outs=[cc_out[:].opt()],
                )
                nc.sync.dma_start(g2[:], cc_out[:].partition_broadcast(P))
            else:
                # local-only min/max (debug)
                dr2 = drp.tile([1, 2], f32)
                nc.sync.dma_start(dr2[:], red2[:])
                nc.sync.dma_start(g2[:], dr2[:].partition_broadcast(P))

            # scale = 1/(max-min) when max-min > EPS else 0
            delta = sm.tile([P, 1], f32)
            nc.vector.tensor_tensor(delta[:], g2[:, 0:1], g2[:, 1:2], op=OP.add)
            maskd = sm.tile([P, 1], f32)
            nc.vector.tensor_scalar(maskd[:], delta[:], EPS, None, op0=OP.is_gt)
            deltac = sm.tile([P, 1], f32)
            nc.vector.tensor_scalar_max(deltac[:], delta[:], EPS)
            sca = sm.tile([P, 1], f32)
            nc.vector.reciprocal(sca[:], deltac[:])
            nc.vector.tensor_tensor(sca[:], sca[:], maskd[:], op=OP.mult)

            # ---- normalize, noise-scale, output ----
            for b, (n0, pb) in enumerate(BLK):
                normb = sm.tile([P, 1], f32)
                # (inf + (-min)) * scale   [g2[:,1:2] holds -min]
                nc.vector.scalar_tensor_tensor(
                    normb[:pb], inf_cols[:pb, b:b + 1], g2[:pb, 1:2], sca[:pb],
                    op0=OP.add, op1=OP.mult,
                )
                stdb = sm.tile([P, 1], f32)
                nc.vector.tensor_scalar(stdb[:pb], normb[:pb],
                                        NOISE_MAX - NOISE_MIN, NOISE_MIN,
                                        op0=OP.mult, op1=OP.add)
                nzb = wk.tile([P, C], f32, tag="nzb")
                nc.sync.dma_start(nzb[:pb], noise[n0:n0 + pb, :])
                fqb2 = wk.tile([P, C], f32, tag="fqb2")
                nc.sync.dma_start(fqb2[:pb], fq[n0:n0 + pb, :])
                ox = wk.tile([P, C], f32, tag="ox")
                # noise*std + f
                nc.vector.scalar_tensor_tensor(
                    ox[:pb], nzb[:pb], stdb[:pb], fqb2[:pb],
                    op0=OP.mult, op1=OP.add,
                )
                nc.sync.dma_start(out_x[n0:n0 + pb, :], ox[:pb])
                nc.sync.dma_start(out_i[n0:n0 + pb, :], normb[:pb])
                nc.sync.dma_start(out_s[n0:n0 + pb, :], stdb[:pb])

    nc.compile()
    _CACHE[("nc", stage)] = nc
    return nc


def _round11(x):
    """Round f32 to 11 explicit mantissa bits (f32r operand precision)."""
    xi = np.ascontiguousarray(x).view(np.uint32)
    shift = 23 - 11
    bias = ((xi >> shift) & 1) + (1 << (shift - 1)) - 1
    return (((xi + bias) >> shift) << shift).astype(np.uint32).view(np.float32)


def prepare_in_maps(features, memory_bank, noise_all):
    features = np.ascontiguousarray(features, dtype=np.float32)
    memory_bank = np.ascontiguousarray(memory_bank, dtype=np.float32)
    f_t = features.reshape(B, C, NQ)            # per-image f^T (C, HW)

    mbt_np = np.zeros((CE, M), dtype=np.float32)
    mbt_np[:C] = memory_bank.T
    mnorm = (memory_bank * memory_bank).sum(axis=1, dtype=np.float32)
    mh = _round11(mnorm)
    mbt_np[C] = -mh
    mbt_np[C + 1] = -(mnorm - mh)

    in_maps = []
    for c in range(N_CORES):
        ft_np = np.zeros((CE, NQ), dtype=np.float32)
        ft_np[:C] = 2.0 * f_t[c]
        ft_np[C] = 1.0
        ft_np[C + 1] = 1.0
        in_maps.append({
            "ft": ft_np,
            "mbt": mbt_np,
            "fq": np.ascontiguousarray(f_t[c].T),
            "mbrows": memory_bank,
            "noise": noise_all[c * NQ:(c + 1) * NQ],
        })
    return in_maps


def assemble_outputs(results):
    noised = np.empty((B, C, H, W), dtype=np.float32)
    inf_norm = np.empty((B, H, W), dtype=np.float32)
    noise_std = np.empty((B, H, W), dtype=np.float32)
    for c in range(N_CORES):
        r = results[c]
        noised[c] = r["out_noised"].reshape(H, W, C).transpose(2, 0, 1)
        inf_norm[c] = r["out_inf"].reshape(H, W)
        noise_std[c] = r["out_std"].reshape(H, W)
    return noised, inf_norm, noise_std


def make_noise():
    """Same expression as the reference; same process/config -> same values."""
    import jax
    import jax.numpy as jnp
    return np.asarray(
        jax.random.normal(jax.random.key(1), (B * H * W, C), dtype=jnp.float32)
    )


def kernel(features, memory_bank):
    from concourse.bass_utils import run_bass_kernel_spmd

    nc = build_bass()
    in_maps = prepare_in_maps(features, memory_bank, make_noise())
    res = run_bass_kernel_spmd(nc, in_maps, list(range(N_CORES)))
    return assemble_outputs(res.results)


# revision 21
# speedup vs baseline: 1.1687x; 1.1687x over previous
"""Trainium2 Bass kernel for AdaptiveNoisingModule (retrieval kNN).

Math (matches the jax reference):
  f = features (B,C,H,W) -> (B*H*W, C) query rows
  d2[n,m] = |f_n|^2 + |mb_m|^2 - 2 f_n.mb_m ; nearest = argmin_m d2
  influence[n] = mean_c |f_n - mb_nearest| / (sqrt(clip(d2min,0)+1e-8) + 1e-8)
  influence_norm = (influence - min) / (max - min)   [global min/max]
  noise_std = 0.01 + 0.49 * influence_norm
  noised = f + noise * noise_std      (noise = jax.random.normal(key(1)))

Sharding: data-parallel over queries; batch image i -> core i (784 rows each).
Memory bank replicated. Only the global min/max of influence is all-reduced.

Device algorithm per core:
  score[n,m] = 2 f.mb - |mb|^2  computed as ONE f32r matmul by extending the
  contraction dim: lhsT rows 0..1023 = 2 f^T, rows 1024/1025 = 1.0;
  rhs rows 0..1023 = mb^T, row 1024 = -mnorm_hi, row 1025 = -mnorm_lo
  (hi/lo split because f32r rounds operands to 11 mantissa bits).
  argmin_m d2 = argmax_m score. Row-local argmax via DVE MAX8/MAX_INDEX over
  1024-wide m-groups, then a tiny cross-group combine. Nearest rows fetched
  with an indirect (gather) DMA; influence on DVE/ACT; min/max AllReduce
  across the 8 cores; noising fused with scalar_tensor_tensor.
"""

import numpy as np

B, C, H, W = 8, 1024, 28, 28
M = 16384
N_CORES = 8
NQ = H * W          # 784 queries per core (batch-sharded)
CE = 1024           # contraction dim (= C)
KB = CE // 128      # 8 k-blocks
G = 16              # m-groups
GW = M // G         # 1024 group width
NT = GW // 512      # psum tiles per group
P = 128
NB = (NQ + P - 1) // P          # 7 row blocks (6x128 + 16)
BLK = [(i * P, min(P, NQ - i * P)) for i in range(NB)]
EPS = 1e-8
NOISE_MIN, NOISE_MAX = 0.01, 0.5
BIG = 3.0e7

_CACHE = {}


def build_bass(stage=4):
    """Build the SPMD Bass graph (same program for all 8 cores).

    stage: debug bisection level — 1: matmul+argmax only, 2: +gather+influence,
    3: +collective, 4: full (normalize+noise outputs).
    """
    if ("nc", stage) in _CACHE:
        return _CACHE[("nc", stage)]
    from contextlib import ExitStack
    import concourse.bass as bass
    import concourse.tile as tile
    from concourse import bacc, mybir

    f32 = mybir.dt.float32
    f32r = mybir.dt.float32r
    u32 = mybir.dt.uint32
    AX = mybir.AxisListType
    OP = mybir.AluOpType
    ACTF = mybir.ActivationFunctionType

    nc = bacc.Bacc("TRN2", target_bir_lowering=False, debug=False,
                   num_devices=N_CORES)

    ft = nc.declare_dram_parameter("ft", [CE, NQ], f32r, isOutput=False)
    mbt = nc.declare_dram_parameter("mbt", [CE, M], f32r, isOutput=False)
    mnorm = nc.declare_dram_parameter("mnorm", [1, M], f32, isOutput=False)
    fq = nc.declare_dram_parameter("fq", [NQ, C], f32, isOutput=False)
    mbrows = nc.declare_dram_parameter("mbrows", [M, C], f32, isOutput=False)
    noise = nc.declare_dram_parameter("noise", [NQ, C], f32, isOutput=False)
    out_x = nc.declare_dram_parameter("out_noised", [NQ, C], f32, isOutput=True)
    out_i = nc.declare_dram_parameter("out_inf", [NQ, 1], f32, isOutput=True)
    out_s = nc.declare_dram_parameter("out_std", [NQ, 1], f32, isOutput=True)

    with tile.TileContext(nc) as tc, ExitStack() as ctx:
        const = ctx.enter_context(tc.tile_pool(name="const", bufs=1))
        mbtp = ctx.enter_context(tc.tile_pool(name="mbtp", bufs=2))
        scp = ctx.enter_context(tc.tile_pool(name="scores", bufs=3))
        psp = ctx.enter_context(tc.tile_pool(name="psum", bufs=4, space="PSUM"))
        wk = ctx.enter_context(tc.tile_pool(name="work", bufs=2))
        sm = ctx.enter_context(tc.tile_pool(name="small", bufs=2))
        drp = ctx.enter_context(tc.tile_pool(name="dram", bufs=1, space="DRAM"))

        # ---- resident tiles ----
        ft_sb = const.tile([P, KB, NQ], f32r)
        nc.sync.dma_start(ft_sb[:], ft.rearrange("(kb p) n -> p kb n", p=P))

        gval = const.tile([P, NB, G, 8], f32)   # top-8 scores per (block, group)
        gidx = const.tile([P, NB, G, 8], u32)   # their in-group indices

        goff = const.tile([P, G], f32)          # column g -> g*GW
        for g in range(G):
            nc.vector.memset(goff[:, g:g + 1], float(g * GW))
        bigc = const.tile([P, G], f32)
        nc.vector.memset(bigc[:], BIG)

        epsc = const.tile([P, 1], f32)
        nc.vector.memset(epsc[:], EPS)

        inf_cols = const.tile([P, NB], f32)     # influence per block column
        infmin = const.tile([P, NB], f32)       # +BIG filled (for min)
        infmax = const.tile([P, NB], f32)       # -BIG filled (for max)
        nc.vector.memset(infmin[:], BIG)
        nc.vector.memset(infmax[:], -BIG)

        # queries + noise resident: prefetched while matmuls run
        fqr = const.tile([P, NB, C], f32)
        nzr = const.tile([P, NB, C], f32)
        for b, (n0, pb) in enumerate(BLK):
            nc.sync.dma_start(fqr[:pb, b, :], fq[n0:n0 + pb, :])
            nc.sync.dma_start(nzr[:pb, b, :], noise[n0:n0 + pb, :])

        # ---- main loop: scores + per-group argmax ----
        for g in range(G):
            mbt_g = mbtp.tile([P, KB, GW], f32r)
            nc.sync.dma_start(
                mbt_g[:],
                mbt[:, g * GW:(g + 1) * GW].rearrange("(kb p) m -> p kb m", p=P),
            )
            mnb = mbtp.tile([P, GW], f32, tag="mnb")
            nc.sync.dma_start(
                mnb[:], mnorm[0:1, g * GW:(g + 1) * GW].partition_broadcast(P)
            )
            for b, (n0, pb) in enumerate(BLK):
                sc = scp.tile([P, GW], f32)
                for t in range(NT):
                    ps = psp.tile([P, 512], f32)
                    for kb in range(KB):
                        nc.tensor.matmul(
                            ps[:pb],
                            ft_sb[:, kb, n0:n0 + pb],
                            mbt_g[:, kb, t * 512:(t + 1) * 512],
                            start=(kb == 0),
                            stop=(kb == KB - 1),
                        )
                    # score = 2 f.mb - mnorm, fused with the PSUM->SBUF move
                    nc.vector.tensor_tensor(
                        sc[:pb, t * 512:(t + 1) * 512], ps[:pb],
                        mnb[:pb, t * 512:(t + 1) * 512], op=OP.subtract,
                    )
                nc.vector.max(gval[:pb, b, g, :], sc[:pb, :])
                nc.vector.max_index(gidx[:pb, b, g, :], gval[:pb, b, g, :],
                                    sc[:pb, :])

        # ---- per-block: combine groups, gather nearest, influence ----
        for b, (n0, pb) in enumerate(BLK if stage >= 2 else []):
            vals = gval[:pb, b, :, 0]            # [pb, G] stride-8
            maxv = sm.tile([P, 1], f32)
            nc.vector.tensor_reduce(maxv[:pb], vals, axis=AX.X, op=OP.max)
            mask = sm.tile([P, G], u32)
            nc.vector.tensor_scalar(mask[:pb], vals, maxv[:pb], None,
                                    op0=OP.is_ge)
            idxf = sm.tile([P, G], f32)
            nc.vector.tensor_copy(idxf[:pb], gidx[:pb, b, :, 0])  # u32 -> f32
            nc.vector.tensor_tensor(idxf[:pb], idxf[:pb], goff[:pb], op=OP.add)
            cand = sm.tile([P, G], f32)
            nc.vector.select(cand[:pb], mask[:pb], idxf[:pb], bigc[:pb])
            idxm = sm.tile([P, 1], f32)
            nc.vector.tensor_reduce(idxm[:pb], cand[:pb], axis=AX.X, op=OP.min)
            idxu = sm.tile([P, 1], u32)
            nc.vector.tensor_copy(idxu[:pb], idxm[:pb])           # f32 -> u32

            nn = wk.tile([P, C], f32, tag="nn")
            nc.gpsimd.indirect_dma_start(
                nn[:pb], None, mbrows[:],
                bass.IndirectOffsetOnAxis(ap=idxu[:pb], axis=0),
            )
            fqb = fqr[:, b, :]

            if stage == 2:
                dbg = wk.tile([P, C], f32, tag="ox")
                nc.vector.memset(dbg[:], 0.0)
                nc.vector.tensor_copy(dbg[:pb, 0:1], idxm[:pb])
                nc.vector.tensor_copy(dbg[:pb, 1:2], maxv[:pb])
                nc.vector.tensor_copy(dbg[:pb, 8:8 + 64], nn[:pb, 0:64])
                nc.sync.dma_start(out_x[n0:n0 + pb, :], dbg[:pb])
                continue

            scr = wk.tile([P, C], f32, tag="scr")
            qn = sm.tile([P, 1], f32)
            nc.scalar.activation(scr[:pb], fqb[:pb], ACTF.Square,
                                 accum_out=qn[:pb])
            if stage == 21:
                dbg = wk.tile([P, C], f32, tag="ox")
                nc.vector.memset(dbg[:], 0.0)
                nc.vector.tensor_copy(dbg[:pb, 0:1], qn[:pb])
                nc.sync.dma_start(out_x[n0:n0 + pb, :], dbg[:pb])
                continue
            diff = wk.tile([P, C], f32, tag="diff")
            nc.vector.tensor_tensor(diff[:pb], fqb[:pb], nn[:pb], op=OP.subtract)
            asum = sm.tile([P, 1], f32)
            nc.vector.tensor_reduce(asum[:pb], diff[:pb], axis=AX.X, op=OP.add,
                                    apply_absolute_value=True)
            if stage == 22:
                dbg = wk.tile([P, C], f32, tag="ox")
                nc.vector.memset(dbg[:], 0.0)
                nc.vector.tensor_copy(dbg[:pb, 0:1], qn[:pb])
                nc.vector.tensor_copy(dbg[:pb, 1:2], asum[:pb])
                nc.sync.dma_start(out_x[n0:n0 + pb, :], dbg[:pb])
                continue
            d2 = sm.tile([P, 1], f32)
            nc.vector.tensor_tensor(d2[:pb], qn[:pb], maxv[:pb], op=OP.subtract)
            nc.vector.tensor_scalar_max(d2[:pb], d2[:pb], 0.0)
            dist = sm.tile([P, 1], f32)
            nc.scalar.activation(dist[:pb], d2[:pb], ACTF.Sqrt, bias=epsc[:pb])
            den = sm.tile([P, 1], f32)
            nc.vector.tensor_scalar_add(den[:pb], dist[:pb], EPS)
            rden = sm.tile([P, 1], f32)
            nc.vector.reciprocal(rden[:pb], den[:pb])
            infb = sm.tile([P, 1], f32)
            nc.vector.tensor_tensor(infb[:pb], asum[:pb], rden[:pb], op=OP.mult)
            nc.vector.tensor_scalar_mul(infb[:pb], infb[:pb], 1.0 / C)
            if stage == 23:
                dbg = wk.tile([P, C], f32, tag="ox")
                nc.vector.memset(dbg[:], 0.0)
                nc.vector.tensor_copy(dbg[:pb, 0:1], infb[:pb])
                nc.sync.dma_start(out_x[n0:n0 + pb, :], dbg[:pb])
                continue
            nc.vector.tensor_copy(inf_cols[:pb, b:b + 1], infb[:pb])
            nc.vector.tensor_copy(infmin[:pb, b:b + 1], infb[:pb])
            nc.vector.tensor_copy(infmax[:pb, b:b + 1], infb[:pb])

        if stage < 2:
            # debug: just dump raw top scores/indices
            for b, (n0, pb) in enumerate(BLK):
                dbg = wk.tile([P, C], f32, tag="ox")
                nc.vector.memset(dbg[:], 0.0)
                nc.vector.tensor_copy(dbg[:pb, 0:G * 8],
                                      gval[:pb, b, :, :].rearrange("p g e -> p (g e)"))
                nc.vector.tensor_copy(dbg[:pb, 256:256 + G * 8],
                                      gidx[:pb, b, :, :].rearrange("p g e -> p (g e)"))
                nc.sync.dma_start(out_x[n0:n0 + pb, :], dbg[:pb])

        if stage in (3, 4):
            # ---- global min/max: partition-reduce via DRAM, AllReduce ----
            vmax = sm.tile([P, 1], f32)
            nc.vector.tensor_reduce(vmax[:], infmax[:], axis=AX.X, op=OP.max)
            vmin = sm.tile([P, 1], f32)
            nc.vector.tensor_reduce(vmin[:], infmin[:], axis=AX.X, op=OP.min)
            pk = sm.tile([P, 2], f32)
            nc.vector.tensor_copy(pk[:, 0:1], vmax[:])
            nc.vector.tensor_scalar_mul(pk[:, 1:2], vmin[:], -1.0)  # -min

            dr_pack = drp.tile([1, 2 * P], f32)
            nc.sync.dma_start(dr_pack[:].rearrange("o (p c) -> (o p) c", c=2),
                              pk[:])
            flat = sm.tile([1, 2 * P], f32)
            nc.sync.dma_start(flat[:], dr_pack[:])
            red2 = sm.tile([1, 2], f32)
            nc.vector.tensor_reduce(
                red2[:], flat[0:1].rearrange("o (p c) -> o c p", c=2),
                axis=AX.X, op=OP.max,
            )
            g2 = sm.tile([P, 2], f32)
            if stage >= 4:
                cc_in = drp.tile([1, 2], f32)
                nc.sync.dma_start(cc_in[:], red2[:])
                cc_out = drp.tile([1, 2], f32, addr_space="Shared")
                nc.gpsimd.collective_compute(
                    "AllReduce", OP.max,
                    replica_groups=[list(range(N_CORES))],
                    ins=[cc_in[:].opt()],
                    outs=[cc_out[:].opt()],
                )
                nc.sync.dma_start(g2[:], cc_out[:].partition_broadcast(P))
            else:
                # local-only min/max (debug)
                dr2 = drp.tile([1, 2], f32)
                nc.sync.dma_start(dr2[:], red2[:])
                nc.sync.dma_start(g2[:], dr2[:].partition_broadcast(P))

            # scale = 1/(max-min) when max-min > EPS else 0
            delta = sm.tile([P, 1], f32)
            nc.vector.tensor_tensor(delta[:], g2[:, 0:1], g2[:, 1:2], op=OP.add)
            maskd = sm.tile([P, 1], f32)
            nc.vector.tensor_scalar(maskd[:], delta[:], EPS, None, op0=OP.is_gt)
            deltac = sm.tile([P, 1], f32)
            nc.vector.tensor_scalar_max(deltac[:], delta[:], EPS)
            sca = sm.tile([P, 1], f32)
            nc.vector.reciprocal(sca[:], deltac[:])
            nc.vector.tensor_tensor(sca[:], sca[:], maskd[:], op=OP.mult)

            # ---- normalize, noise-scale, output ----
            for b, (n0, pb) in enumerate(BLK):
                normb = sm.tile([P, 1], f32)
                # (inf + (-min)) * scale   [g2[:,1:2] holds -min]
                nc.vector.scalar_tensor_tensor(
                    normb[:pb], inf_cols[:pb, b:b + 1], g2[:pb, 1:2], sca[:pb],
                    op0=OP.add, op1=OP.mult,
                )
                stdb = sm.tile([P, 1], f32)
                nc.vector.tensor_scalar(stdb[:pb], normb[:pb],
                                        NOISE_MAX - NOISE_MIN, NOISE_MIN,
                                        op0=OP.mult, op1=OP.add)
                ox = wk.tile([P, C], f32, tag="ox")
                # noise*std + f
                nc.vector.scalar_tensor_tensor(
                    ox[:pb], nzr[:, b, :][:pb], stdb[:pb], fqr[:, b, :][:pb],
                    op0=OP.mult, op1=OP.add,
                )
                nc.sync.dma_start(out_x[n0:n0 + pb, :], ox[:pb])
                nc.sync.dma_start(out_i[n0:n0 + pb, :], normb[:pb])
                nc.sync.dma_start(out_s[n0:n0 + pb, :], stdb[:pb])

    nc.compile()
    _CACHE[("nc", stage)] = nc
    return nc


def _round11(x):
    """Round f32 to 11 explicit mantissa bits (f32r operand precision)."""
    xi = np.ascontiguousarray(x).view(np.uint32)
    shift = 23 - 11
    bias = ((xi >> shift) & 1) + (1 << (shift - 1)) - 1
    return (((xi + bias) >> shift) << shift).astype(np.uint32).view(np.float32)


def prepare_in_maps(features, memory_bank, noise_all):
    features = np.ascontiguousarray(features, dtype=np.float32)
    memory_bank = np.ascontiguousarray(memory_bank, dtype=np.float32)
    f_t = features.reshape(B, C, NQ)            # per-image f^T (C, HW)

    mbt_np = np.ascontiguousarray(memory_bank.T)
    mnorm_np = (memory_bank * memory_bank).sum(axis=1, dtype=np.float32)[None, :]

    in_maps = []
    for c in range(N_CORES):
        in_maps.append({
            "ft": np.ascontiguousarray(2.0 * f_t[c]),
            "mbt": mbt_np,
            "mnorm": mnorm_np,
            "fq": np.ascontiguousarray(f_t[c].T),
            "mbrows": memory_bank,
            "noise": np.ascontiguousarray(noise_all[c * NQ:(c + 1) * NQ]),
        })
    return in_maps


def assemble_outputs(results):
    noised = np.empty((B, C, H, W), dtype=np.float32)
    inf_norm = np.empty((B, H, W), dtype=np.float32)
    noise_std = np.empty((B, H, W), dtype=np.float32)
    for c in range(N_CORES):
        r = results[c]
        noised[c] = r["out_noised"].reshape(H, W, C).transpose(2, 0, 1)
        inf_norm[c] = r["out_inf"].reshape(H, W)
        noise_std[c] = r["out_std"].reshape(H, W)
    return noised, inf_norm, noise_std


def make_noise():
    """Same expression as the reference; same process/config -> same values."""
    import jax
    import jax.numpy as jnp
    return np.asarray(
        jax.random.normal(jax.random.key(1), (B * H * W, C), dtype=jnp.float32)
    )


def kernel(features, memory_bank):
    from concourse.bass_utils import run_bass_kernel_spmd

    nc = build_bass()
    in_maps = prepare_in_maps(features, memory_bank, make_noise())
    res = run_bass_kernel_spmd(nc, in_maps, list(range(N_CORES)))
    return assemble_outputs(res.results)
